# revision 1
# baseline (speedup 1.0000x reference)
"""Trainium2 Bass kernel for CustomConv2d:
  x [16, 32, 512, 512] f32, weight [32, 32, 3, 3] f32, bias [32] f32
  -> out [16, 32, 510, 510] f32   (stride 1, VALID padding, + bias)

Data-parallel over batch: 2 images per core across 8 NeuronCores.

v2 design (pair layouts for 4KB HBM DMA descriptors):
 - x SBUF layout: partition 32g+ci holds image row r (g=(r//2)%4), free
   offset 1024*(r//8 - slot0) + 512*(r%2). Input DMA descriptors cover 2
   consecutive rows (4KB contiguous HBM).
 - block = 8 output rows (y0 % 8 == 0): row p -> PSUM partitions 32*(p//2),
   sub-slot p%2. Output partition 32j+co holds rows y0+2j, y0+2j+1 at
   consecutive free offsets -> 4080B-contiguous output descriptors.
 - conv: per row 2-3 bf16 K<=64 matmuls on the 64x32-tiled PE (tile A =
   partitions 0-63, B = 64-127), grouped by (tile, row parity, slot).
   Banks are tile-pure: (sub, A), (sub, B); empty (bank, col) groups get a
   single zero-weight matmul so drains stay uniform [128-partition].
 - drain: ScalarE Identity(bankB + bias) -> t; VectorE t + bankA -> ostrip.
 - weights: one contiguous DMA + DVE 32x32 transposes + 48 small SBUF->SBUF
   block copies build 22 distinct [64, 32] variants; cast once to bf16.
"""
import numpy as np

import concourse.bass as bass
import concourse.tile as tile
from concourse import bacc, mybir
from concourse.bass_utils import run_bass_kernel_spmd
from contextlib import ExitStack

F32 = mybir.dt.float32
BF16 = mybir.dt.bfloat16

N_FULL, C, H, W = 16, 32, 512, 512
HO = WO = 510
N_CORES = 8
N_PER = N_FULL // N_CORES
N_STRIPS = H // 32                 # 16 strips of 32 rows
XCOLS = 4 * 1024                   # strip tile free size (4 slots-of-8)


def _mm_table():
    """Per phase p (row offset in block, 0..7): list of MMs
    (T, o, sl, items) with items = [(gi, kh)], grouped by (T, o, sl)."""
    table = []
    for p in range(8):
        groups = {}
        for kh in range(3):
            r = p + kh
            g = (r // 2) % 4
            T = 0 if g < 2 else 1
            groups.setdefault((T, r % 2, r // 8), []).append((g % 2, kh))
        table.append([(T, o, sl, tuple(items))
                      for (T, o, sl), items in groups.items()])
    return table

MM_TABLE = _mm_table()

# distinct variant contents: key = tuple(sorted(items)); plus the zero key ()
_KEYS = sorted({tuple(sorted(it)) for row in MM_TABLE for (_, _, _, it) in row})
# column layout: for kw in 0..2: one col per key; then one shared zero col
WCOL = {}
for k in _KEYS:
    for kw in range(3):
        WCOL[(k, kw)] = len(WCOL)
ZCOL = len(WCOL)
NWCOL = ZCOL + 1                    # 22 columns of 32


def _build():
    nc = bacc.Bacc("TRN2", target_bir_lowering=False, debug=False, num_devices=1)
    x_d = nc.dram_tensor("x", [N_PER, C, H, W], F32, kind="ExternalInput").ap()
    w_d = nc.dram_tensor("w", [C, C, 3, 3], F32, kind="ExternalInput").ap()
    b_d = nc.dram_tensor("b", [C], F32, kind="ExternalInput").ap()
    o_d = nc.dram_tensor("out", [N_PER, C, HO, WO], F32, kind="ExternalOutput").ap()

    with tile.TileContext(nc) as tc, ExitStack() as ctx:
        const_pool = ctx.enter_context(tc.tile_pool(name="const", bufs=1))
        xf_pool = ctx.enter_context(tc.tile_pool(name="xf", bufs=2))
        xb_pool = ctx.enter_context(tc.tile_pool(name="xb", bufs=3))
        psum_pool = ctx.enter_context(tc.tile_pool(name="ps", bufs=2, space="PSUM"))
        t_pool = ctx.enter_context(tc.tile_pool(name="t", bufs=4))
        out_pool = ctx.enter_context(tc.tile_pool(name="ostrip", bufs=3))

        # ---- weights: contiguous load + on-chip transpose + block copies ----
        # wstage[co, ci*9 + kh*3 + kw]
        wstage = const_pool.tile([32, 288], F32)
        nc.sync.dma_start(wstage[:], w_d[:].rearrange("o i h w -> o (i h w)"))
        # wT[ci, 32*(kh*3+kw) + co] via 9 DVE 32x32 block transposes
        wg = const_pool.tile([32, 288], F32)
        for t9 in range(9):
            nc.vector.tensor_copy(wg[:, 32 * t9:32 * t9 + 32],
                                  wstage[:, t9:288:9])
        wT = const_pool.tile([32, 288], F32)
        nc.vector.transpose(wT[:], wg[:])
        # variant image [128, NWCOL*32] fp32: A copy at partitions 0-63,
        # B copy at 64-127 (same contents)
        wf = const_pool.tile([128, NWCOL * 32], F32)
        nc.vector.memset(wf[:], 0.0)
        for key in _KEYS:
            c0 = WCOL[(key, 0)] * 32          # 3 kw cols are adjacent
            for gi, kh in key:
                for base in (0, 64):
                    nc.sync.dma_start(
                        wf[base + 32 * gi:base + 32 * gi + 32, c0:c0 + 96],
                        wT[:, 96 * kh:96 * kh + 96])
        wb = const_pool.tile([128, NWCOL * 32], BF16)
        nc.vector.tensor_copy(wb[:], wf[:])
        bt = const_pool.tile([128, 1], F32)
        for j in range(4):
            nc.gpsimd.dma_start(bt[32 * j:32 * j + 32, 0:1], b_d[:].unsqueeze(1))

        # queue discipline: input slot DMAs alternate the two HWDGE queues
        # (sync/scalar); output block DMAs ride the async SWDGE (gpsimd)
        # so they never head-of-line-block the ACT drain stream.
        uid = [0]
        in_rr = [0]

        def in_dma(dst, src):
            eng = nc.scalar if in_rr[0] % 2 == 0 else nc.gpsimd
            in_rr[0] += 1
            eng.dma_start(dst, src)

        def out_dma(dst, src):
            nc.sync.dma_start(dst, src)

        def emit_block(xb_cur, xb_next, b8, ostrip, nrow=8):
            """MMs + drain for one block (nrow output rows, y0 = 32s+8*b8).
            Block uses slots b8, b8+1 of the strip tile; slot 4 -> xb_next
            slot 0."""
            psA = {}
            psB = {}
            for sub in range(2):
                uid[0] += 1
                psA[sub] = psum_pool.tile([128, 512], F32, tag=f"psA{sub}",
                                          name=f"psA{sub}_{uid[0]}")
                psB[sub] = psum_pool.tile([128, 512], F32, tag=f"psB{sub}",
                                          name=f"psB{sub}_{uid[0]}")
            banks = {0: psA, 1: psB}
            njs = (nrow + 1) // 2
            # collect MM lists per (T, sub, j)
            groups = {}
            for kw in range(3):
                for p in range(nrow):
                    j, sub = p // 2, p % 2
                    for (T, o, sl, items) in MM_TABLE[p]:
                        col = WCOL[(tuple(sorted(items)), kw)]
                        groups.setdefault((T, sub, j), []).append(
                            (o, sl, col, kw))
            # zero-pad empty (T, sub, j) combos so drains are uniform
            for sub in range(2):
                for j in range(njs):
                    for T in range(2):
                        if (T, sub, j) not in groups:
                            groups[(T, sub, j)] = [(0, 0, ZCOL, 0)]
            # emit: kw-major over the collected lists, preserving per-group
            # order for start/stop flags
            idx = {k: 0 for k in groups}
            order = []
            for k, mms in groups.items():
                for i, m in enumerate(mms):
                    order.append((m[3], k, i, m))
            order.sort(key=lambda e: (e[0], e[2], e[1][2], e[1][1], e[1][0]))
            for _, (T, sub, j), i, (o, sl, col, kw) in order:
                ps = banks[T][sub]
                xa = xb_cur
                if sl + b8 >= 4:
                    xa = xb_next
                soff = (sl + b8) % 4 * 1024 + 512 * o + kw
                p0 = 0 if T == 0 else 64
                n_mms = len(groups[(T, sub, j)])
                nc.tensor.matmul(
                    ps[32 * j:32 * j + 32, 0:WO],
                    wb[p0:p0 + 64, 32 * col:32 * col + 32],
                    xa[p0:p0 + 64, soff:soff + WO],
                    start=(i == 0), stop=(i == n_mms - 1),
                    skip_group_check=True,
                    tile_position=(p0, 32 * j),
                )
            # drains: per sub, ACT Identity(B + bias) then DVE + A
            npart = 32 * njs
            for sub in range(2):
                uid[0] += 1
                t = t_pool.tile([128, WO], F32, tag="t", name=f"t_{uid[0]}")
                nc.scalar.activation(t[0:npart, :], psB[sub][0:npart, 0:WO],
                                     mybir.ActivationFunctionType.Identity,
                                     bias=bt[0:npart, :])
                nc.vector.tensor_add(
                    ostrip[0:npart, b8 * 1020 + 510 * sub:
                           b8 * 1020 + 510 * sub + WO],
                    t[0:npart, :], psA[sub][0:npart, 0:WO])

        def dma_out_strip(n, s, ostrip):
            # pairs of rows per descriptor: rows 32s + 8*b8 + 2j + e
            nb8 = 4 if s < N_STRIPS - 1 else 3
            rows = o_d[n, :, 32 * s:32 * s + 8 * nb8, :].rearrange(
                "c (b8 j2 e) w -> j2 c b8 (e w)", b8=nb8, j2=4, e=2)
            for j in range(4):
                out_dma(rows[j], ostrip[32 * j:32 * j + 32, 0:nb8 * 1020])
            if s == N_STRIPS - 1:
                for j in range(3):
                    dst = o_d[n, :, 504 + 2 * j:506 + 2 * j, :].rearrange(
                        "c e w -> c (e w)")
                    out_dma(dst, ostrip[32 * j:32 * j + 32,
                                        3 * 1020:3 * 1020 + 1020])

        for n in range(N_PER):
            prev = None  # (xb_prev, ostrip_prev, strip_idx)
            for s in range(N_STRIPS):
                uid[0] += 1
                xf = xf_pool.tile([128, XCOLS], F32, tag="xf",
                                  name=f"xf_{uid[0]}")
                xsrc = x_d[n, :, 32 * s:32 * s + 32, :].rearrange(
                    "c (t g2 e) w -> g2 c t (e w)", t=4, g2=4, e=2)
                for g in range(4):
                    in_dma(xf[32 * g:32 * g + 32, :], xsrc[g])
                xb = xb_pool.tile([128, XCOLS], BF16, tag="xb",
                                  name=f"xb_{uid[0]}")
                nc.vector.tensor_copy(xb[:], xf[:])

                if prev is not None:
                    xbp, osp, sp = prev
                    emit_block(xbp, xb, 3, osp)
                    dma_out_strip(n, sp, osp)
                uid[0] += 1
                ostrip = out_pool.tile([128, 4 * 1020], F32, tag="ostrip",
                                       name=f"os_{uid[0]}")
                if s < N_STRIPS - 1:
                    for b8 in range(3):
                        emit_block(xb, None, b8, ostrip)
                    prev = (xb, ostrip, s)
                else:
                    for b8 in range(3):
                        emit_block(xb, None, b8, ostrip)
                    emit_block(xb, None, 3, ostrip, nrow=6)
                    dma_out_strip(n, s, ostrip)
                    prev = None

    nc.compile()
    return nc


_NC = None


def kernel(x, weight, bias):
    global _NC
    x = np.ascontiguousarray(np.asarray(x, dtype=np.float32))
    weight = np.ascontiguousarray(np.asarray(weight, dtype=np.float32))
    bias = np.ascontiguousarray(np.asarray(bias, dtype=np.float32))
    if _NC is None:
        _NC = _build()
    in_maps = [
        {"x": x[N_PER * i:N_PER * (i + 1)], "w": weight, "b": bias}
        for i in range(N_CORES)
    ]
    res = run_bass_kernel_spmd(_NC, in_maps, core_ids=list(range(N_CORES)))
    return np.concatenate([r["out"] for r in res.results], axis=0)

